# revision 53
# baseline (speedup 1.0000x reference)
"""Trainium2 Bass kernel for GQA attention (B=2, S=2048, DIM=2048, H=16, KV=8,
HD=128) with RoPE + causal mask + output projection.

Sharding: 8-way tensor parallelism over heads. Core c computes q heads
{2c, 2c+1} and kv head c end-to-end (QKV projection, RoPE, causal attention),
contributes its transposed attention output to on-device AllGathers, then
computes the output-projection column slice out[:, 256c:256(c+1)] from the
gathered activations. The host only slices inputs and concatenates outputs.

Design notes (v3 583us -> 484us, measured on the axon trn2 fleet):
The PE on this part sustains only ~1.2GHz under load (throttle active most
of the run), so matmul instruction COUNT is the currency (each [*,512]
matmul ~425ns, ~240ns effective when back-to-back). Everything that is not
a real contraction runs off the PE, and every engine's per-j-iteration work
stays below the PE's ~850ns (scores+AV):
- RoPE: m1 = src*[cos;sin], m2 = src*[sin;cos] (2 wide DVE muls; the DVE
  reads the PSUM projection accumulators directly), assembly
  (even = m1[r]-m1[r+64], odd = m2[r]+m2[r+64]) via two PE matmuls against
  a constant +-identity matrix (deferred one window so the PE never waits
  on the DVE), one ACT evict per unit. The BIR verifier forbids DVE
  tensor-tensor with two SBUF operands at different base partitions, which
  rules out doing the assembly on the DVE.
- causal mask: 0/1 band multiply on the DVE after exp (exact zeros), no PE
  mask matmuls; diagonal blocks are column-trimmed in scores/exp/sum/AV.
- softmax denominator: DVE accumulates exp tiles into eacc (bf16, SBUF);
  finalize = PE ones-matmul (denominator row) -> DVE
  reciprocal_approx_fast -> PE row-broadcast matmul -> ACT evict -> DVE
  normalize mul. The chain is PE/DVE/ACT only (~4us): no DMA, no gpsimd,
  so attention never transitively waits on the collective stream (variants
  that put finalize work on the gpsimd queue or on broadcast DMAs lost
  40us+ stuck behind running AllGathers). pav is triple-buffered (psV
  bufs=3) so finalize latency up to ~2 passes is absorbed; the gpsimd
  queue carries only fire-and-forget ag stores, collective triggers, and
  gathered-chunk loads.
- attention runs per (chunk, head) pass, j ascending; AV pipelined 3
  behind exp. Batches stay sequential: interleaving them delays the first
  AllGather past ~200us of accumulated inter-core skew, and the first
  collective after the entry barrier absorbs ALL of that skew (measured up
  to 80us), so early AllGathers must fire early.
- wo chunks interleave into later attention emission, always >= 3 chunks
  behind their AllGather (robust to a slow first collective); gathered
  chunks load with ONE dma each on the gpsimd queue; x windows load in
  512KB sub-DMAs on sync; qkv weights + rope tables load from the scalar
  queue (the gpsimd queue sits behind the kernel-entry collective
  barrier), wo weights last.
- the last chunk's AllGather is split per head so its wo can start ~12us
  earlier (even feature blocks consume the h0 gather while h1 flies).
"""

import sys

if "/opt/trn_rl_repo" not in sys.path:
    sys.path.insert(0, "/opt/trn_rl_repo")

import numpy as np
import ml_dtypes

B, S, DIM = 2, 2048, 2048
H, KV, HD = 16, 8, 128
NC = 8
NS = B * S            # 4096 flattened (b, s) rows
P = 128
MB = DIM // P         # 16 contraction blocks for the projections
BF = ml_dtypes.bfloat16

_cache: dict = {}


def _build(debug=False):
    import concourse.bass as bass
    import concourse.bass_isa as bass_isa
    import concourse.mybir as mybir
    import concourse.tile as tile
    from concourse import bacc
    from concourse.masks import make_identity

    dt = mybir.dt
    f32, bf16 = dt.float32, dt.bfloat16
    Exp = mybir.ActivationFunctionType.Exp

    nc = bacc.Bacc("TRN2", debug=False, target_bir_lowering=False, num_devices=NC)

    # x^T pre-tiled as [window, 128, MB*512]: one contiguous 2MB DMA per window
    xT_h = nc.dram_tensor("xT", (8, P, MB * 512), bf16, kind="ExternalInput").ap()
    wq_h = nc.dram_tensor("wq_c", (P, MB * 256), bf16, kind="ExternalInput").ap()
    wk_h = nc.dram_tensor("wk_c", (P, MB * HD), bf16, kind="ExternalInput").ap()
    wv_h = nc.dram_tensor("wv_c", (P, MB * HD), bf16, kind="ExternalInput").ap()
    wo_h = nc.dram_tensor("wo_c", (P, MB * 256), bf16, kind="ExternalInput").ap()
    # stacked rope tables: cs = [cos;sin], sc = [sin;cos]  (128, NS)
    cs_h = nc.dram_tensor("csT", (P, NS), bf16, kind="ExternalInput").ap()
    sc_h = nc.dram_tensor("scT", (P, NS), bf16, kind="ExternalInput").ap()
    # rope assembly matrices: cols 0:64 -> even out = m1[r]-m1[r+64],
    # cols 64:128 -> odd out = m2[r]+m2[r+64]
    asm_h = nc.dram_tensor("asmc", (P, P), bf16, kind="ExternalInput").ap()
    # one [128,128] causal band bias (-30 above the diagonal)
    mskb_h = nc.dram_tensor("maskb", (P, P), bf16, kind="ExternalInput").ap()
    out_h = nc.dram_tensor("outT", (256, NS), f32, kind="ExternalOutput").ap()
    dbg = {}
    if debug:
        for nm, shp in [("qrot_d", (P, 2 * NS)), ("krot_d", (P, NS)),
                        ("vnat_d", (P, NS)), ("oav_d", (P, 2 * NS)),
                        ("ag_d", (NC * 256, NS))]:
            dbg[nm] = nc.dram_tensor(nm, shp, bf16, kind="ExternalOutput").ap()

    with tile.TileContext(nc) as tc:
        with (
            tc.tile_pool(name="const", bufs=1) as const,
            tc.tile_pool(name="persist", bufs=1) as persist,
            tc.tile_pool(name="xs", bufs=2) as xs,
            tc.tile_pool(name="tmp", bufs=3) as tmp,
            tc.tile_pool(name="et", bufs=6) as et,
            tc.tile_pool(name="gp", bufs=2) as gp,
            tc.tile_pool(name="ot", bufs=3) as ot,
            tc.tile_pool(name="dram", bufs=1, space="DRAM") as dram,
        ):
            # ---- constants into SBUF. qkv weights + rope tables load from
            # the SCALAR queue in 4-m-block chunks: the gpsimd queue sits
            # behind the kernel-entry collective barrier (~20us), scalar
            # does not. wo loads last on gpsimd (first needed ~200us in).
            wq_sb = const.tile([P, MB, 256], bf16)
            wk_sb = const.tile([P, MB, HD], bf16)
            wv_sb = const.tile([P, MB, HD], bf16)
            for mq in range(0, MB, 4):
                ms = slice(mq, mq + 4)
                for sb, hh, d in ((wq_sb, wq_h, 256), (wk_sb, wk_h, HD),
                                  (wv_sb, wv_h, HD)):
                    nc.scalar.dma_start(
                        sb[:, ms, :],
                        hh.rearrange("p (mb d) -> p mb d", mb=MB)[:, ms, :])
            cs_sb = const.tile([P, NS], bf16)
            nc.scalar.dma_start(cs_sb[:], cs_h)
            sc_sb = const.tile([P, NS], bf16)
            nc.scalar.dma_start(sc_sb[:], sc_h)
            mskb_sb = const.tile([P, P], bf16)
            nc.scalar.dma_start(mskb_sb[:], mskb_h)
            asm_sb = const.tile([P, P], bf16)
            nc.scalar.dma_start(asm_sb[:], asm_h)
            wo_sb = const.tile([P, MB, 256], bf16)
            nc.gpsimd.dma_start(wo_sb[:], wo_h.rearrange("p (mb d) -> p mb d", mb=MB))
            ones_sb = const.tile([P, 1], bf16)
            nc.gpsimd.memset(ones_sb[:], 1.0)
            ones_row = const.tile([1, P], bf16)
            nc.gpsimd.memset(ones_row[:], 1.0)
            ident = const.tile([P, P], bf16)
            make_identity(nc, ident[:])

            # ---- per-batch persistent activations ----
            qrot = [persist.tile([P, 2, S], bf16, name=f"qrot{b}") for b in range(B)]
            krot = [persist.tile([P, S], bf16, name=f"krot{b}") for b in range(B)]
            vTt = [persist.tile([P, S], bf16, name=f"vTt{b}") for b in range(B)]
            vnat = [persist.tile([P, S // P, HD], bf16, name=f"vnat{b}")
                    for b in range(B)]
            oav = [persist.tile([P, 2, S], bf16, name=f"oav{b}") for b in range(B)]
            ag_in = [[dram.tile([256, 512], bf16, name=f"agi{b}{t}")
                      for t in range(4)] for b in range(B)]
            ag_out = [[dram.tile([NC * 256, 512], bf16, name=f"ago{b}{t}")
                       for t in range(4)] for b in range(B)]
            # per-head gathers for the very last chunk (b=1, t=3)
            ag_outh = [dram.tile([NC * P, 512], bf16, name=f"agoh{h}")
                       for h in range(2)]

            def emit_proj(b):
                """QKV projections (transposed layout) + RoPE for batch b.
                The rope assembly (PE matmuls vs asm_sb + ACT evict) for
                window sp is deferred until after window sp+1's projection
                matmuls so the PE never waits on the DVE table-muls."""
                with tc.tile_pool(name=f"psA{b}", bufs=2, space="PSUM") as psA:
                    deferred = []

                    def rope_muls(src, gw):
                        # src: [128,512] PSUM f32 with rows [evens;odds]
                        m1 = tmp.tile([P, 512], bf16, tag="r1", name="m1",
                                      bufs=6)
                        m2 = tmp.tile([P, 512], bf16, tag="r2", name="m2",
                                      bufs=6)
                        nc.vector.tensor_mul(m1[:], src[:], cs_sb[:, gw])
                        nc.vector.tensor_mul(m2[:], src[:], sc_sb[:, gw])
                        return m1, m2

                    def rope_asm(units):
                        for m1, m2, dst in units:
                            a = psA.tile([P, 512], f32, tag="asm", name="a",
                                         bufs=1)
                            nc.tensor.matmul(a[0:64, :], asm_sb[:, 0:64],
                                             m1[:], start=True, stop=True)
                            nc.tensor.matmul(a[64:128, :], asm_sb[:, 64:128],
                                             m2[:], start=True, stop=True)
                            nc.scalar.copy(dst, a[:])

                    for sp in range(4):          # 512-col windows within batch
                        w = b * 4 + sp
                        gw = slice(w * 512, (w + 1) * 512)
                        lw = slice(sp * 512, (sp + 1) * 512)
                        xw = xs.tile([P, MB, 512], bf16, tag="xw", name="xw")
                        # 4 sub-loads so the first m-block matmuls can start
                        # ~5us in instead of waiting for the whole 2MB window
                        xsrc = xT_h[w].rearrange("p (mb c) -> p mb c", mb=MB)
                        for mq in range(0, MB, 4):
                            nc.sync.dma_start(xw[:, mq:mq + 4, :],
                                              xsrc[:, mq:mq + 4, :])
                        pq = [psA.tile([P, 512], f32, tag=f"pq{h}", name=f"pq{h}")
                              for h in range(2)]
                        pk = psA.tile([P, 512], f32, tag="pk", name="pk")
                        pv = psA.tile([P, 512], f32, tag="pv", name="pv", bufs=1)
                        for m in range(MB):
                            for acc, lhsT in (
                                (pq[0], wq_sb[:, m, 0:128]),
                                (pq[1], wq_sb[:, m, 128:256]),
                                (pv, wv_sb[:, m, :]),
                                (pk, wk_sb[:, m, :]),
                            ):
                                nc.tensor.matmul(
                                    acc[:], lhsT, xw[:, m, :],
                                    start=(m == 0), stop=(m == MB - 1),
                                )
                        # pv is single-buffered: evict it first
                        nc.scalar.copy(vTt[b][:, lw], pv[:])
                        units = [rope_muls(pk, gw)
                                 + (krot[b][:, lw],)]
                        for h in range(2):
                            units.append(rope_muls(pq[h], gw)
                                         + (qrot[b][:, h, lw],))
                        for fn in deferred:
                            fn()
                        deferred = [lambda u=units: rope_asm(u)]
                    for fn in deferred:
                        fn()

                # ---- v natural layout via PE transposes ----
                with tc.tile_pool(name=f"psT{b}", bufs=2, space="PSUM") as psT:
                    for blk in range(S // P):
                        pt = psT.tile([P, P], bf16, tag="pt", name="pt")
                        nc.tensor.transpose(
                            pt[:], vTt[b][:, blk * P:(blk + 1) * P], ident[:])
                        nc.scalar.copy(vnat[b][:, blk, :], pt[:])

            # ---- wo output projection for gathered 512-col chunks ----
            def emit_wo_chunks(b, ts, g, split=False):
                rorder = ([0, 2, 4, 6, 8, 10, 12, 14, 1, 3, 5, 7, 9, 11, 13, 15]
                          if split else list(range(MB)))
                with tc.tile_pool(name=f"psW{b}{ts[0]}", bufs=1,
                                  space="PSUM") as psW:
                    for ti, t in enumerate(ts):
                        cs = slice(ti * 512, (ti + 1) * 512)
                        for n in range(2):
                            pw = psW.tile([P, 512], f32, tag=f"pw{n}",
                                          name=f"pw{n}")
                            for i, r in enumerate(rorder):
                                nc.tensor.matmul(
                                    pw[:],
                                    wo_sb[:, r, n * 128:(n + 1) * 128],
                                    g[:, r, cs],
                                    start=(i == 0), stop=(i == MB - 1),
                                    skip_group_check=True,
                                )
                            o = ot.tile([P, 512], f32, tag="o", name="o")
                            nc.scalar.copy(o[:], pw[:])
                            nc.sync.dma_start(
                                out_h[n * P:(n + 1) * P,
                                      b * S + t * 512: b * S + (t + 1) * 512],
                                o[:],
                            )

            def emit_wo(b, t, split=False):
                g = gp.tile([P, MB, 512], bf16, tag="g", name="g")
                if not split:
                    # two half-loads so the first 8 r-block matmuls start
                    # while the second half is still in flight
                    gsrc = ag_out[b][t].rearrange("(r p) q -> p r q", p=P)
                    for rq in range(0, MB, 8):
                        nc.gpsimd.dma_start(g[:, rq:rq + 8, :],
                                            gsrc[:, rq:rq + 8, :])
                    emit_wo_chunks(b, (t,), g)
                    return
                if split:
                    # per-head gathers: even feature blocks r=2c from the h0
                    # gather, odd from h1; even-r matmuls run while the h1
                    # gather is still in flight
                    for hh in range(2):
                        nc.gpsimd.dma_start(
                            g[:, hh::2, :],
                            ag_outh[hh].rearrange("(c p) q -> p c q", p=P))
                else:
                    nc.gpsimd.dma_start(
                        g[:], ag_out[b][t].rearrange("(r p) q -> p r q", p=P))
                emit_wo_chunks(b, (t,), g, split=split)

            wo_queue = []

            def emit_attn(b, allow_wo):
                """Causal attention in scoresT layout, one (chunk, head) pass
                at a time; deferred finalizes; lagged wo chunks. Chunks t0+t1
                share one AllGather (fewer ops on the serial cc stream)."""
                with (
                    tc.tile_pool(name=f"psS{b}", bufs=2, space="PSUM") as psS,
                    tc.tile_pool(name=f"psV{b}", bufs=3, space="PSUM") as psV,
                    tc.tile_pool(name=f"psD{b}", bufs=1, space="PSUM") as psD,
                ):
                    def finalize(fin):
                        """Normalize a finished (t,h) pass and fire its
                        AllGather. Deferred one pass; PE/DVE only (~4us),
                        nothing here ever waits on the collective stream."""
                        pav_f, ecast_f, t_f, h_f = fin
                        il_f = slice(t_f * 512, (t_f + 1) * 512)
                        pden = psD.tile([1, 512], f32, tag="pd", name="pd")
                        nc.tensor.matmul(pden[:], ones_sb[:], ecast_f[:],
                                         start=True, stop=True)
                        rcp = tmp.tile([1, 512], f32, tag="rcp", name="rcp")
                        nc.vector.reciprocal_approx_fast(rcp[:], pden[:])
                        rcp_bf = tmp.tile([1, 512], bf16, tag="rcpc",
                                          name="rcpc")
                        nc.vector.tensor_copy(rcp_bf[:], rcp[:])
                        rb = psS.tile([P, 512], f32, tag="ps", name="rb")
                        nc.tensor.matmul(rb[:], ones_row[:], rcp_bf[:],
                                         start=True, stop=True)
                        rcp_b = tmp.tile([P, 512], f32, tag="rcpb", name="rcpb")
                        nc.scalar.copy(rcp_b[:], rb[:])
                        nc.vector.tensor_mul(oav[b][:, h_f, il_f],
                                             pav_f[:], rcp_b[:])
                        nc.gpsimd.dma_start(
                            ag_in[b][t_f][h_f * P:(h_f + 1) * P, :],
                            oav[b][:, h_f, il_f],
                        )
                        if h_f == 1:
                            nc.gpsimd.collective_compute(
                                "AllGather",
                                mybir.AluOpType.bypass,
                                replica_groups=[list(range(NC))],
                                ins=[ag_in[b][t_f].opt()],
                                outs=[ag_out[b][t_f].opt()],
                            )

                    pending = None
                    for t in range(4):            # query chunks of 512
                        nj = 4 * t + 4
                        for h in range(2):
                            pav = psV.tile([P, 512], f32, tag="pav", name="pav")
                            eacc = tmp.tile([P, 512], bf16, tag="eacc",
                                            name="eacc", bufs=2)
                            pipe = []
                            for j in range(nj):
                                rel = j - 4 * t
                                cl = 0 if rel < 0 else rel * P
                                ps = psS.tile([P, 512], f32, tag="ps", name="ps")
                                nc.tensor.matmul(
                                    ps[:, cl:512],
                                    krot[b][:, j * P:(j + 1) * P],
                                    qrot[b][:, h, t * 512 + cl:(t + 1) * 512],
                                    start=True, stop=True,
                                )
                                e = et.tile([P, 512], bf16, tag="e", name="e")
                                nc.scalar.activation(e[:, cl:512], ps[:, cl:512],
                                                     Exp)
                                if rel >= 0:
                                    # causal 0/1 band mask: exact zeros above
                                    # the diagonal of the transition band
                                    nc.vector.tensor_mul(
                                        e[:, cl:cl + P], e[:, cl:cl + P],
                                        mskb_sb[:])
                                if j == 0:
                                    nc.vector.tensor_copy(eacc[:], e[:])
                                else:
                                    nc.vector.tensor_add(
                                        eacc[:, cl:512], eacc[:, cl:512],
                                        e[:, cl:512])
                                pipe.append((e, cl, j))
                                if j == 1 and pending is not None:
                                    finalize(pending)
                                    pending = None
                                if j == 2 and h == 1 and allow_wo and wo_queue:
                                    emit_wo(*wo_queue.pop(0))
                                if len(pipe) > 4:
                                    ep, cp, jp = pipe.pop(0)
                                    nc.tensor.matmul(
                                        pav[:, cp:512], vnat[b][:, jp, :],
                                        ep[:, cp:512],
                                        start=(jp == 0), stop=(jp == nj - 1),
                                        skip_group_check=True,
                                    )
                            for ep, cp, jp in pipe:
                                nc.tensor.matmul(
                                    pav[:, cp:512], vnat[b][:, jp, :],
                                    ep[:, cp:512],
                                    start=(jp == 0), stop=(jp == nj - 1),
                                    skip_group_check=True,
                                )
                            pending = (pav, eacc, t, h)
                        if t >= 2:
                            wo_queue.append((b, t - 2))
                    finalize(pending)

            # ---- global schedule ----
            emit_proj(0)
            emit_attn(0, allow_wo=False)       # queues (0,0), (0,1)
            emit_proj(1)
            wo_queue.append((0, 2))
            wo_queue.append((0, 3))
            # attn b1 starts immediately (its gather stream is the critical
            # path at the end); all four b0 wo chunks pop inside it
            emit_attn(1, allow_wo=True)        # pops (0,0)..(0,3) at t0..t3
            wo_queue.append((1, 2))
            while wo_queue:                    # wo(1,0), wo(1,1), wo(1,2)
                emit_wo(*wo_queue.pop(0))
            emit_wo(1, 3)

            if debug:
                for b in range(B):
                    for h in range(2):
                        nc.sync.dma_start(
                            dbg["qrot_d"][:, h * NS + b * S: h * NS + (b + 1) * S],
                            qrot[b][:, h, :])
                        nc.sync.dma_start(
                            dbg["oav_d"][:, h * NS + b * S: h * NS + (b + 1) * S],
                            oav[b][:, h, :])
                    nc.sync.dma_start(dbg["krot_d"][:, b * S:(b + 1) * S], krot[b][:])
                    nc.sync.dma_start(
                        dbg["vnat_d"].rearrange("p (bb d) -> p bb d", bb=NS // P)
                        [:, b * (S // P):(b + 1) * (S // P), :], vnat[b][:])
                    for t in range(4):
                        nc.sync.dma_start(
                            dbg["ag_d"][:, b * S + t * 512: b * S + (t + 1) * 512],
                            ag_out[b][t][:])

    nc.compile()
    return nc


def _prep_inputs(x, freqs_cos, freqs_sin, wq, wk, wv, wo):
    x = np.asarray(x, np.float32).reshape(NS, DIM)
    # [window, p, mb, 512] so each window is one contiguous 2MB block
    xT = np.ascontiguousarray(
        x.reshape(8, 512, MB, P).transpose(0, 3, 2, 1)).astype(BF)
    cos = np.asarray(freqs_cos, np.float32)
    sin = np.asarray(freqs_sin, np.float32)
    cosT = np.tile(cos, (B, 1)).T            # (64, NS)
    sinT = np.tile(sin, (B, 1)).T
    csT = np.ascontiguousarray(np.concatenate([cosT, sinT], 0)).astype(BF)
    scT = np.ascontiguousarray(np.concatenate([sinT, cosT], 0)).astype(BF)

    perm = np.r_[np.arange(0, HD, 2), np.arange(1, HD, 2)]
    scale = np.float32(1.0 / np.sqrt(HD))
    wq = np.asarray(wq, np.float32) * scale
    wk = np.asarray(wk, np.float32)
    wv = np.asarray(wv, np.float32)
    wo = np.asarray(wo, np.float32)

    # [128,128] causal 0/1 band mask: 1 where key-row p <= query-col q
    maskb = np.ascontiguousarray(np.triu(np.ones((P, P), np.float32))).astype(BF)
    # rope assembly matrices (lhsT layout [contraction p, out r]):
    # even: out[r] = m1[r] - m1[r+64]; odd: out[r] = m2[r] + m2[r+64]
    eye64 = np.eye(64, dtype=np.float32)
    asmc = np.zeros((P, P), np.float32)
    asmc[0:64, 0:64] = eye64
    asmc[64:128, 0:64] = -eye64
    asmc[0:64, 64:128] = eye64
    asmc[64:128, 64:128] = eye64
    asmc = np.ascontiguousarray(asmc).astype(BF)

    def tile_w(w):
        # (2048, d) -> (128, 16*d): row mi holds [mb, d] contiguously
        d = w.shape[1]
        return np.ascontiguousarray(
            w.reshape(MB, P, d).transpose(1, 0, 2).reshape(P, MB * d)).astype(BF)

    in_maps = []
    for c in range(NC):
        wq_c = wq[:, c * 256:(c + 1) * 256]
        wq_cp = np.concatenate([wq_c[:, h * HD + perm] for h in range(2)], axis=1)
        in_maps.append({
            "xT": xT.reshape(8, P, MB * 512),
            "wq_c": tile_w(wq_cp),
            "wk_c": tile_w(wk[:, c * HD:(c + 1) * HD][:, perm]),
            "wv_c": tile_w(wv[:, c * HD:(c + 1) * HD]),
            "wo_c": tile_w(wo[:, c * 256:(c + 1) * 256]),
            "csT": csT,
            "scT": scT,
            "maskb": maskb,
            "asmc": asmc,
        })
    return in_maps


def _run(inputs, trace=False, **kw):
    from concourse.bass_utils import run_bass_kernel_spmd

    if "nc" not in _cache:
        _cache["nc"] = _build()
    nc = _cache["nc"]
    in_maps = _prep_inputs(**inputs)
    res = run_bass_kernel_spmd(
        nc, in_maps, core_ids=list(range(NC)), trace=trace, **kw
    )
    out = np.empty((NS, DIM), np.float32)
    for c in range(NC):
        out[:, c * 256:(c + 1) * 256] = res.results[c]["outT"].T
    return out.reshape(B, S, DIM), res


def kernel(**inputs) -> np.ndarray:
    out, _ = _run(inputs, trace=False)
    return out


# revision 59
# speedup vs baseline: 1.0848x; 1.0848x over previous
"""Trainium2 Bass kernel for GQA attention (B=2, S=2048, DIM=2048, H=16, KV=8,
HD=128) with RoPE + causal mask + output projection.

Sharding: 8-way tensor parallelism over heads. Core c computes q heads
{2c, 2c+1} and kv head c end-to-end (QKV projection, RoPE, causal attention),
contributes its transposed attention output to on-device AllGathers, then
computes the output-projection column slice out[:, 256c:256(c+1)] from the
gathered activations. The host only slices inputs and concatenates outputs.

Design notes (v3 583us -> 484us, measured on the axon trn2 fleet):
The PE on this part sustains only ~1.2GHz under load (throttle active most
of the run), so matmul instruction COUNT is the currency (each [*,512]
matmul ~425ns, ~240ns effective when back-to-back). Everything that is not
a real contraction runs off the PE, and every engine's per-j-iteration work
stays below the PE's ~850ns (scores+AV):
- RoPE: m1 = src*[cos;sin], m2 = src*[sin;cos] (2 wide DVE muls; the DVE
  reads the PSUM projection accumulators directly), assembly
  (even = m1[r]-m1[r+64], odd = m2[r]+m2[r+64]) via two PE matmuls against
  a constant +-identity matrix (deferred one window so the PE never waits
  on the DVE), one ACT evict per unit. The BIR verifier forbids DVE
  tensor-tensor with two SBUF operands at different base partitions, which
  rules out doing the assembly on the DVE.
- causal mask: 0/1 band multiply on the DVE after exp (exact zeros), no PE
  mask matmuls; diagonal blocks are column-trimmed in scores/exp/sum/AV.
- softmax denominator: DVE accumulates exp tiles into eacc (bf16, SBUF);
  finalize = PE ones-matmul (denominator row) -> DVE
  reciprocal_approx_fast -> PE row-broadcast matmul -> ACT evict -> DVE
  normalize mul. The chain is PE/DVE/ACT only (~4us): no DMA, no gpsimd,
  so attention never transitively waits on the collective stream (variants
  that put finalize work on the gpsimd queue or on broadcast DMAs lost
  40us+ stuck behind running AllGathers). pav is triple-buffered (psV
  bufs=3) so finalize latency up to ~2 passes is absorbed; the gpsimd
  queue carries only fire-and-forget ag stores, collective triggers, and
  gathered-chunk loads.
- attention runs per (chunk, head) pass, j ascending; AV pipelined 3
  behind exp. Batches stay sequential: interleaving them delays the first
  AllGather past ~200us of accumulated inter-core skew, and the first
  collective after the entry barrier absorbs ALL of that skew (measured up
  to 80us), so early AllGathers must fire early.
- wo chunks interleave into later attention emission, always >= 3 chunks
  behind their AllGather (robust to a slow first collective); gathered
  chunks load with ONE dma each on the gpsimd queue; x windows load in
  512KB sub-DMAs on sync; qkv weights + rope tables load from the scalar
  queue (the gpsimd queue sits behind the kernel-entry collective
  barrier), wo weights last.
- the last chunk's AllGather is split per head so its wo can start ~12us
  earlier (even feature blocks consume the h0 gather while h1 flies).
"""

import sys

if "/opt/trn_rl_repo" not in sys.path:
    sys.path.insert(0, "/opt/trn_rl_repo")

import numpy as np
import ml_dtypes

B, S, DIM = 2, 2048, 2048
H, KV, HD = 16, 8, 128
NC = 8
NS = B * S            # 4096 flattened (b, s) rows
P = 128
MB = DIM // P         # 16 contraction blocks for the projections
BF = ml_dtypes.bfloat16

_cache: dict = {}


def _build(debug=False):
    import concourse.bass as bass
    import concourse.bass_isa as bass_isa
    import concourse.mybir as mybir
    import concourse.tile as tile
    from concourse import bacc
    from concourse.masks import make_identity

    dt = mybir.dt
    f32, bf16 = dt.float32, dt.bfloat16
    Exp = mybir.ActivationFunctionType.Exp

    nc = bacc.Bacc("TRN2", debug=False, target_bir_lowering=False, num_devices=NC)

    # x^T pre-tiled as [window, 128, MB*512]: one contiguous 2MB DMA per window
    xT_h = nc.dram_tensor("xT", (8, P, MB * 512), bf16, kind="ExternalInput").ap()
    wq_h = nc.dram_tensor("wq_c", (P, MB * 256), bf16, kind="ExternalInput").ap()
    wk_h = nc.dram_tensor("wk_c", (P, MB * HD), bf16, kind="ExternalInput").ap()
    wv_h = nc.dram_tensor("wv_c", (P, MB * HD), bf16, kind="ExternalInput").ap()
    wo_h = nc.dram_tensor("wo_c", (P, MB * 256), bf16, kind="ExternalInput").ap()
    # stacked rope tables: cs = [cos;sin], sc = [sin;cos]  (128, NS)
    cs_h = nc.dram_tensor("csT", (P, NS), bf16, kind="ExternalInput").ap()
    sc_h = nc.dram_tensor("scT", (P, NS), bf16, kind="ExternalInput").ap()
    # rope assembly matrices: cols 0:64 -> even out = m1[r]-m1[r+64],
    # cols 64:128 -> odd out = m2[r]+m2[r+64]
    asm_h = nc.dram_tensor("asmc", (P, P), bf16, kind="ExternalInput").ap()
    # one [128,128] causal band bias (-30 above the diagonal)
    mskb_h = nc.dram_tensor("maskb", (P, P), bf16, kind="ExternalInput").ap()
    out_h = nc.dram_tensor("outT", (256, NS), bf16, kind="ExternalOutput").ap()
    dbg = {}
    if debug:
        for nm, shp in [("qrot_d", (P, 2 * NS)), ("krot_d", (P, NS)),
                        ("vnat_d", (P, NS)), ("oav_d", (P, 2 * NS)),
                        ("ag_d", (NC * 256, NS))]:
            dbg[nm] = nc.dram_tensor(nm, shp, bf16, kind="ExternalOutput").ap()

    with tile.TileContext(nc) as tc:
        with (
            tc.tile_pool(name="const", bufs=1) as const,
            tc.tile_pool(name="persist", bufs=1) as persist,
            tc.tile_pool(name="xs", bufs=2) as xs,
            tc.tile_pool(name="tmp", bufs=3) as tmp,
            tc.tile_pool(name="et", bufs=6) as et,
            tc.tile_pool(name="gp", bufs=2) as gp,
            tc.tile_pool(name="ot", bufs=3) as ot,
            tc.tile_pool(name="dram", bufs=1, space="DRAM") as dram,
        ):
            # ---- constants into SBUF. qkv weights + rope tables load from
            # the SCALAR queue in 4-m-block chunks: the gpsimd queue sits
            # behind the kernel-entry collective barrier (~20us), scalar
            # does not. wo loads last on gpsimd (first needed ~200us in).
            wq_sb = const.tile([P, MB, 256], bf16)
            wk_sb = const.tile([P, MB, HD], bf16)
            wv_sb = const.tile([P, MB, HD], bf16)
            for mq in range(0, MB, 4):
                ms = slice(mq, mq + 4)
                for sb, hh, d in ((wq_sb, wq_h, 256), (wk_sb, wk_h, HD),
                                  (wv_sb, wv_h, HD)):
                    nc.scalar.dma_start(
                        sb[:, ms, :],
                        hh.rearrange("p (mb d) -> p mb d", mb=MB)[:, ms, :])
            cs_sb = const.tile([P, NS], bf16)
            nc.scalar.dma_start(cs_sb[:], cs_h)
            sc_sb = const.tile([P, NS], bf16)
            nc.scalar.dma_start(sc_sb[:], sc_h)
            mskb_sb = const.tile([P, P], bf16)
            nc.scalar.dma_start(mskb_sb[:], mskb_h)
            asm_sb = const.tile([P, P], bf16)
            nc.scalar.dma_start(asm_sb[:], asm_h)
            wo_sb = const.tile([P, MB, 256], bf16)
            nc.gpsimd.dma_start(wo_sb[:], wo_h.rearrange("p (mb d) -> p mb d", mb=MB))
            ones_sb = const.tile([P, 1], bf16)
            nc.gpsimd.memset(ones_sb[:], 1.0)
            ones_row = const.tile([1, P], bf16)
            nc.gpsimd.memset(ones_row[:], 1.0)
            ident = const.tile([P, P], bf16)
            make_identity(nc, ident[:])

            # ---- per-batch persistent activations ----
            qrot = [persist.tile([P, 2, S], bf16, name=f"qrot{b}") for b in range(B)]
            krot = [persist.tile([P, S], bf16, name=f"krot{b}") for b in range(B)]
            vTt = [persist.tile([P, S], bf16, name=f"vTt{b}") for b in range(B)]
            vnat = [persist.tile([P, S // P, HD], bf16, name=f"vnat{b}")
                    for b in range(B)]
            oav = [persist.tile([P, 2, S], bf16, name=f"oav{b}") for b in range(B)]
            ag_in = [[dram.tile([256, 512], bf16, name=f"agi{b}{t}")
                      for t in range(4)] for b in range(B)]
            ag_out = [[dram.tile([NC * 256, 512], bf16, name=f"ago{b}{t}")
                       for t in range(4)] for b in range(B)]
            # per-head gathers for the very last chunk (b=1, t=3)
            ag_outh = [dram.tile([NC * P, 512], bf16, name=f"agoh{h}")
                       for h in range(2)]

            def emit_proj(b):
                """QKV projections (transposed layout) + RoPE for batch b.
                The rope assembly (PE matmuls vs asm_sb + ACT evict) for
                window sp is deferred until after window sp+1's projection
                matmuls so the PE never waits on the DVE table-muls."""
                with tc.tile_pool(name=f"psA{b}", bufs=2, space="PSUM") as psA:
                    deferred = []

                    def rope_muls(src, gw):
                        # src: [128,512] PSUM f32 with rows [evens;odds]
                        m1 = tmp.tile([P, 512], bf16, tag="r1", name="m1",
                                      bufs=6)
                        m2 = tmp.tile([P, 512], bf16, tag="r2", name="m2",
                                      bufs=6)
                        nc.vector.tensor_mul(m1[:], src[:], cs_sb[:, gw])
                        nc.vector.tensor_mul(m2[:], src[:], sc_sb[:, gw])
                        return m1, m2

                    def rope_asm(units):
                        for m1, m2, dst in units:
                            a = psA.tile([P, 512], f32, tag="asm", name="a",
                                         bufs=1)
                            nc.tensor.matmul(a[0:64, :], asm_sb[:, 0:64],
                                             m1[:], start=True, stop=True)
                            nc.tensor.matmul(a[64:128, :], asm_sb[:, 64:128],
                                             m2[:], start=True, stop=True)
                            nc.scalar.copy(dst, a[:])

                    for sp in range(4):          # 512-col windows within batch
                        w = b * 4 + sp
                        gw = slice(w * 512, (w + 1) * 512)
                        lw = slice(sp * 512, (sp + 1) * 512)
                        xw = xs.tile([P, MB, 512], bf16, tag="xw", name="xw")
                        # 4 sub-loads so the first m-block matmuls can start
                        # ~5us in instead of waiting for the whole 2MB window
                        xsrc = xT_h[w].rearrange("p (mb c) -> p mb c", mb=MB)
                        for mq in range(0, MB, 4):
                            nc.sync.dma_start(xw[:, mq:mq + 4, :],
                                              xsrc[:, mq:mq + 4, :])
                        pq = [psA.tile([P, 512], f32, tag=f"pq{h}", name=f"pq{h}")
                              for h in range(2)]
                        pk = psA.tile([P, 512], f32, tag="pk", name="pk")
                        pv = psA.tile([P, 512], f32, tag="pv", name="pv", bufs=1)
                        for m in range(MB):
                            for acc, lhsT in (
                                (pq[0], wq_sb[:, m, 0:128]),
                                (pq[1], wq_sb[:, m, 128:256]),
                                (pv, wv_sb[:, m, :]),
                                (pk, wk_sb[:, m, :]),
                            ):
                                nc.tensor.matmul(
                                    acc[:], lhsT, xw[:, m, :],
                                    start=(m == 0), stop=(m == MB - 1),
                                )
                        # pv is single-buffered: evict it first
                        nc.scalar.copy(vTt[b][:, lw], pv[:])
                        units = [rope_muls(pk, gw)
                                 + (krot[b][:, lw],)]
                        for h in range(2):
                            units.append(rope_muls(pq[h], gw)
                                         + (qrot[b][:, h, lw],))
                        for fn in deferred:
                            fn()
                        deferred = [lambda u=units: rope_asm(u)]
                    for fn in deferred:
                        fn()

                # ---- v natural layout via PE transposes ----
                with tc.tile_pool(name=f"psT{b}", bufs=2, space="PSUM") as psT:
                    for blk in range(S // P):
                        pt = psT.tile([P, P], bf16, tag="pt", name="pt")
                        nc.tensor.transpose(
                            pt[:], vTt[b][:, blk * P:(blk + 1) * P], ident[:])
                        nc.scalar.copy(vnat[b][:, blk, :], pt[:])

            # ---- wo output projection for gathered 512-col chunks ----
            def emit_wo_chunks(b, ts, g, split=False):
                rorder = ([0, 2, 4, 6, 8, 10, 12, 14, 1, 3, 5, 7, 9, 11, 13, 15]
                          if split else list(range(MB)))
                with tc.tile_pool(name=f"psW{b}{ts[0]}", bufs=1,
                                  space="PSUM") as psW:
                    for ti, t in enumerate(ts):
                        cs = slice(ti * 512, (ti + 1) * 512)
                        for n in range(2):
                            pw = psW.tile([P, 512], f32, tag=f"pw{n}",
                                          name=f"pw{n}")
                            for i, r in enumerate(rorder):
                                nc.tensor.matmul(
                                    pw[:],
                                    wo_sb[:, r, n * 128:(n + 1) * 128],
                                    g[:, r, cs],
                                    start=(i == 0), stop=(i == MB - 1),
                                    skip_group_check=True,
                                )
                            o = ot.tile([P, 512], bf16, tag="o", name="o")
                            nc.scalar.copy(o[:], pw[:])
                            nc.sync.dma_start(
                                out_h[n * P:(n + 1) * P,
                                      b * S + t * 512: b * S + (t + 1) * 512],
                                o[:],
                            )

            def emit_wo(b, t, split=False, tail=False):
                g = gp.tile([P, MB, 512], bf16, tag="g", name="g")
                if tail:
                    # final chunk is on the post-stream critical chain: two
                    # half-loads so the first 8 r-block matmuls overlap the
                    # second half's transfer
                    gsrc = ag_out[b][t].rearrange("(r p) q -> p r q", p=P)
                    for rq in (0, 8):
                        nc.gpsimd.dma_start(g[:, rq:rq + 8, :],
                                            gsrc[:, rq:rq + 8, :])
                    emit_wo_chunks(b, (t,), g)
                    return
                if split:
                    # per-head gathers: even feature blocks r=2c from the h0
                    # gather, odd from h1; even-r matmuls run while the h1
                    # gather is still in flight
                    for hh in range(2):
                        nc.gpsimd.dma_start(
                            g[:, hh::2, :],
                            ag_outh[hh].rearrange("(c p) q -> p c q", p=P))
                else:
                    nc.gpsimd.dma_start(
                        g[:], ag_out[b][t].rearrange("(r p) q -> p r q", p=P))
                emit_wo_chunks(b, (t,), g, split=split)

            wo_queue = []

            def emit_attn(b, allow_wo):
                """Causal attention in scoresT layout, one (chunk, head) pass
                at a time; deferred finalizes; lagged wo chunks. Chunks t0+t1
                share one AllGather (fewer ops on the serial cc stream)."""
                with (
                    # b0 has no wo pools open: spend the spare bank on a
                    # deeper scores pipeline
                    tc.tile_pool(name=f"psS{b}", bufs=(3 if b == 0 else 2),
                                 space="PSUM") as psS,
                    tc.tile_pool(name=f"psV{b}", bufs=3, space="PSUM") as psV,
                    tc.tile_pool(name=f"psD{b}", bufs=1, space="PSUM") as psD,
                ):
                    def finalize(fin):
                        """Normalize a finished (t,h) pass and fire its
                        AllGather. Deferred one pass; PE/DVE only (~4us),
                        nothing here ever waits on the collective stream."""
                        pav_f, ecast_f, t_f, h_f = fin
                        il_f = slice(t_f * 512, (t_f + 1) * 512)
                        pden = psD.tile([1, 512], f32, tag="pd", name="pd")
                        nc.tensor.matmul(pden[:], ones_sb[:], ecast_f[:],
                                         start=True, stop=True)
                        rcp = tmp.tile([1, 512], f32, tag="rcp", name="rcp")
                        nc.vector.reciprocal_approx_fast(rcp[:], pden[:])
                        rcp_bf = tmp.tile([1, 512], bf16, tag="rcpc",
                                          name="rcpc")
                        nc.vector.tensor_copy(rcp_bf[:], rcp[:])
                        rb = psS.tile([P, 512], f32, tag="ps", name="rb")
                        nc.tensor.matmul(rb[:], ones_row[:], rcp_bf[:],
                                         start=True, stop=True)
                        rcp_b = tmp.tile([P, 512], f32, tag="rcpb", name="rcpb")
                        nc.scalar.copy(rcp_b[:], rb[:])
                        nc.vector.tensor_mul(oav[b][:, h_f, il_f],
                                             pav_f[:], rcp_b[:])
                        nc.gpsimd.dma_start(
                            ag_in[b][t_f][h_f * P:(h_f + 1) * P, :],
                            oav[b][:, h_f, il_f],
                        )
                        if h_f == 1:
                            nc.gpsimd.collective_compute(
                                "AllGather",
                                mybir.AluOpType.bypass,
                                replica_groups=[list(range(NC))],
                                ins=[ag_in[b][t_f].opt()],
                                outs=[ag_out[b][t_f].opt()],
                            )

                    pending = None
                    for t in range(4):            # query chunks of 512
                        nj = 4 * t + 4
                        for h in range(2):
                            pav = psV.tile([P, 512], f32, tag="pav", name="pav")
                            eacc = tmp.tile([P, 512], bf16, tag="eacc",
                                            name="eacc", bufs=2)
                            pipe = []
                            for j in range(nj):
                                rel = j - 4 * t
                                cl = 0 if rel < 0 else rel * P
                                ps = psS.tile([P, 512], f32, tag="ps", name="ps")
                                nc.tensor.matmul(
                                    ps[:, cl:512],
                                    krot[b][:, j * P:(j + 1) * P],
                                    qrot[b][:, h, t * 512 + cl:(t + 1) * 512],
                                    start=True, stop=True,
                                )
                                e = et.tile([P, 512], bf16, tag="e", name="e")
                                nc.scalar.activation(e[:, cl:512], ps[:, cl:512],
                                                     Exp)
                                if rel >= 0:
                                    # causal 0/1 band mask: exact zeros above
                                    # the diagonal of the transition band
                                    nc.vector.tensor_mul(
                                        e[:, cl:cl + P], e[:, cl:cl + P],
                                        mskb_sb[:])
                                if j == 0:
                                    nc.vector.tensor_copy(eacc[:], e[:])
                                else:
                                    nc.vector.tensor_add(
                                        eacc[:, cl:512], eacc[:, cl:512],
                                        e[:, cl:512])
                                pipe.append((e, cl, j))
                                if j == 1 and pending is not None:
                                    finalize(pending)
                                    pending = None
                                if j == 2 and h == 1 and allow_wo and wo_queue:
                                    emit_wo(*wo_queue.pop(0))
                                if len(pipe) > 3:
                                    ep, cp, jp = pipe.pop(0)
                                    nc.tensor.matmul(
                                        pav[:, cp:512], vnat[b][:, jp, :],
                                        ep[:, cp:512],
                                        start=(jp == 0), stop=(jp == nj - 1),
                                        skip_group_check=True,
                                    )
                            for ep, cp, jp in pipe:
                                nc.tensor.matmul(
                                    pav[:, cp:512], vnat[b][:, jp, :],
                                    ep[:, cp:512],
                                    start=(jp == 0), stop=(jp == nj - 1),
                                    skip_group_check=True,
                                )
                            pending = (pav, eacc, t, h)
                        if t >= 2:
                            wo_queue.append((b, t - 2))
                    finalize(pending)

            # ---- global schedule ----
            emit_proj(0)
            emit_attn(0, allow_wo=False)       # queues (0,0), (0,1)
            emit_proj(1)
            wo_queue.append((0, 2))
            wo_queue.append((0, 3))
            # attn b1 starts immediately (its gather stream is the critical
            # path at the end); all four b0 wo chunks pop inside it
            emit_attn(1, allow_wo=True)        # pops (0,0)..(0,3) at t0..t3
            wo_queue.append((1, 2))
            while wo_queue:                    # wo(1,0), wo(1,1), wo(1,2)
                emit_wo(*wo_queue.pop(0))
            emit_wo(1, 3, tail=True)

            if debug:
                for b in range(B):
                    for h in range(2):
                        nc.sync.dma_start(
                            dbg["qrot_d"][:, h * NS + b * S: h * NS + (b + 1) * S],
                            qrot[b][:, h, :])
                        nc.sync.dma_start(
                            dbg["oav_d"][:, h * NS + b * S: h * NS + (b + 1) * S],
                            oav[b][:, h, :])
                    nc.sync.dma_start(dbg["krot_d"][:, b * S:(b + 1) * S], krot[b][:])
                    nc.sync.dma_start(
                        dbg["vnat_d"].rearrange("p (bb d) -> p bb d", bb=NS // P)
                        [:, b * (S // P):(b + 1) * (S // P), :], vnat[b][:])
                    for t in range(4):
                        nc.sync.dma_start(
                            dbg["ag_d"][:, b * S + t * 512: b * S + (t + 1) * 512],
                            ag_out[b][t][:])

    nc.compile()
    return nc


def _prep_inputs(x, freqs_cos, freqs_sin, wq, wk, wv, wo):
    x = np.asarray(x, np.float32).reshape(NS, DIM)
    # [window, p, mb, 512] so each window is one contiguous 2MB block
    xT = np.ascontiguousarray(
        x.reshape(8, 512, MB, P).transpose(0, 3, 2, 1)).astype(BF)
    cos = np.asarray(freqs_cos, np.float32)
    sin = np.asarray(freqs_sin, np.float32)
    cosT = np.tile(cos, (B, 1)).T            # (64, NS)
    sinT = np.tile(sin, (B, 1)).T
    csT = np.ascontiguousarray(np.concatenate([cosT, sinT], 0)).astype(BF)
    scT = np.ascontiguousarray(np.concatenate([sinT, cosT], 0)).astype(BF)

    perm = np.r_[np.arange(0, HD, 2), np.arange(1, HD, 2)]
    scale = np.float32(1.0 / np.sqrt(HD))
    wq = np.asarray(wq, np.float32) * scale
    wk = np.asarray(wk, np.float32)
    wv = np.asarray(wv, np.float32)
    wo = np.asarray(wo, np.float32)

    # [128,128] causal 0/1 band mask: 1 where key-row p <= query-col q
    maskb = np.ascontiguousarray(np.triu(np.ones((P, P), np.float32))).astype(BF)
    # rope assembly matrices (lhsT layout [contraction p, out r]):
    # even: out[r] = m1[r] - m1[r+64]; odd: out[r] = m2[r] + m2[r+64]
    eye64 = np.eye(64, dtype=np.float32)
    asmc = np.zeros((P, P), np.float32)
    asmc[0:64, 0:64] = eye64
    asmc[64:128, 0:64] = -eye64
    asmc[0:64, 64:128] = eye64
    asmc[64:128, 64:128] = eye64
    asmc = np.ascontiguousarray(asmc).astype(BF)

    def tile_w(w):
        # (2048, d) -> (128, 16*d): row mi holds [mb, d] contiguously
        d = w.shape[1]
        return np.ascontiguousarray(
            w.reshape(MB, P, d).transpose(1, 0, 2).reshape(P, MB * d)).astype(BF)

    in_maps = []
    for c in range(NC):
        wq_c = wq[:, c * 256:(c + 1) * 256]
        wq_cp = np.concatenate([wq_c[:, h * HD + perm] for h in range(2)], axis=1)
        in_maps.append({
            "xT": xT.reshape(8, P, MB * 512),
            "wq_c": tile_w(wq_cp),
            "wk_c": tile_w(wk[:, c * HD:(c + 1) * HD][:, perm]),
            "wv_c": tile_w(wv[:, c * HD:(c + 1) * HD]),
            "wo_c": tile_w(wo[:, c * 256:(c + 1) * 256]),
            "csT": csT,
            "scT": scT,
            "maskb": maskb,
            "asmc": asmc,
        })
    return in_maps


def _run(inputs, trace=False, **kw):
    from concourse.bass_utils import run_bass_kernel_spmd

    if "nc" not in _cache:
        _cache["nc"] = _build()
    nc = _cache["nc"]
    in_maps = _prep_inputs(**inputs)
    res = run_bass_kernel_spmd(
        nc, in_maps, core_ids=list(range(NC)), trace=trace, **kw
    )
    out = np.empty((NS, DIM), np.float32)
    for c in range(NC):
        out[:, c * 256:(c + 1) * 256] = res.results[c]["outT"].T
    return out.reshape(B, S, DIM), res


def kernel(**inputs) -> np.ndarray:
    out, _ = _run(inputs, trace=False)
    return out
